# revision 39
# baseline (speedup 1.0000x reference)
"""Trainium2 Bass kernel for nn_BinaryLinear (binarized linear layer).

Computes: out = sign(x) @ sign(W).T + bias
  x: [8192, 4096] f32, W: [4096, 4096] f32, bias: [4096] f32 -> out [8192, 4096] f32
  sign(v) = +1 if v >= 0 else -1

Sharding: 4x2 grid over 8 NeuronCores — batch split 4 ways (2048 rows each),
W rows (out_features) split 2 ways (2048 each). Each core computes a disjoint
[2048, 2048] output block; no collectives. This minimizes per-core input bytes
(32 MiB x-shard + 32 MiB W-shard).

Device-side algorithm (per core):
  1. Prep: DMA f32 tiles in natural layout, transpose 128x128 tiles on the
     TensorEngine (identity matmul), then binarize PSUM -> SBUF fp8 as +-0.5 in
     a single DVE op: (v >= 0 ? 1 : 0) - 0.5. Both operands end up [K-on-
     partitions, rows-on-free] fp8, fully resident in SBUF.
  2. Matmul: standard K-accumulated PE matmuls, psum = (sum of +-0.25 terms)
     = exact_integer_result / 4. fp8 +-0.5 operands are exact, and the fp32
     PSUM accumulation of quarter-integers bounded by 1024 is exact.
  3. Epilogue: ACT copy with scale=4 (psum*4 -> exact integers), DVE add of the
     bias row (pre-replicated across 128 partitions on host), DMA out.
"""

import math
import os

import numpy as np

import concourse.bacc as bacc
import concourse.mybir as mybir
import concourse.tile as tile
from concourse.alu_op_type import AluOpType
from concourse.bass_utils import run_bass_kernel_spmd
from concourse.masks import make_identity

P = 128
N_CORES = 8
M_SPLIT = 4  # batch split
N_SPLIT = 2  # out_features split

# Full-problem shapes (hardcoded per harness contract)
BATCH = 8192
IN_FEATURES = 4096
OUT_FEATURES = 4096

F32 = mybir.dt.float32
FP8 = mybir.dt.float8e4

SUPER = 512  # rows per prep "super" == matmul o-panel width (one PSUM bank)


def build_nc(
    M,
    K,
    N,
    n_cores=N_CORES,
    double_row=True,
    repeat=1,
    timing_variant=False,
    body_parts="all",  # "all" | "mm" | "prep"  (timing ablation)
    prep_mode="fp8t",  # "fp8t": binarize then fp8 transpose; "f32t": f32 transpose then binarize
    cast_loads=True,  # SWDGE cast-DMA loads (f32 -> bf16 during DMA)
    kg=8,  # k-tiles batched per transpose-psum bank / ACT copyback
    tps_bufs=2,
    mm_bufs=6,
    out_bufs=6,
    stage_bufs=3,
    chunk_prep=False,  # split prep loads into KC-column chunks
):
    """Build the per-core kernel: x_shard [M, K], w_shard [N, K],
    bias_rep [P, N] -> out_shard [M, N].

    repeat/timing_variant are for HW timing only (wrap the body in a device-side
    loop; inputs/outputs become internal DRAM so nothing ships over the tunnel).
    The graded kernel() path always uses repeat=1, timing_variant=False.
    """
    assert M % SUPER == 0 and N % SUPER == 0 and K % P == 0
    KT = K // P  # contraction tiles
    M_SUPERS = M // SUPER
    N_SUPERS = N // SUPER
    RB = SUPER // P  # row-blocks per super (4)
    I_CHUNK = min(1024, K)  # staged i-columns per DMA
    ICT = I_CHUNK // P
    if double_row:
        assert KT % 2 == 0

    nc = bacc.Bacc(
        "TRN2", target_bir_lowering=False, debug=False, num_devices=n_cores
    )
    if timing_variant:
        x_in = nc.dram_tensor("x_int", [M, K], F32).ap()
        w_in = nc.dram_tensor("w_int", [N, K], F32).ap()
        b_in = nc.dram_tensor("b_int", [P, N], F32).ap()
        out = nc.dram_tensor("out_int", [M, N], F32).ap()
        dummy_out = nc.dram_tensor("dummy_out", [P, 16], F32, kind="ExternalOutput").ap()
    else:
        x_in = nc.dram_tensor("x_shard", [M, K], F32, kind="ExternalInput").ap()
        w_in = nc.dram_tensor("w_shard", [N, K], F32, kind="ExternalInput").ap()
        b_in = nc.dram_tensor("bias_rep", [P, N], F32, kind="ExternalInput").ap()
        out = nc.dram_tensor("out_shard", [M, N], F32, kind="ExternalOutput").ap()

    with tile.TileContext(nc) as tc:
        with (
            tc.tile_pool(name="const", bufs=1) as const,
            tc.tile_pool(name="resid", bufs=1) as resid,
            tc.tile_pool(name="stage", bufs=stage_bufs) as stage_pool,
            tc.tile_pool(name="tps", bufs=tps_bufs, space="PSUM") as tps_pool,
            tc.tile_pool(name="mm", bufs=mm_bufs, space="PSUM") as mm_pool,
            tc.tile_pool(name="outp", bufs=out_bufs) as out_pool,
        ):
            id_dt = FP8 if prep_mode == "fp8t" else F32
            identity = const.tile([P, P], id_dt, name="identity", tag="identity")
            make_identity(nc, identity)
            bias_sb = const.tile([P, N], F32, name="bias_sb", tag="bias_sb")
            nc.sync.dma_start(bias_sb, b_in)

            xT = [
                resid.tile([P, KT, SUPER], FP8, name=f"xT{s}", tag=f"xT{s}")
                for s in range(M_SUPERS)
            ]
            wT = [
                resid.tile([P, KT, SUPER], FP8, name=f"wT{s}", tag=f"wT{s}")
                for s in range(N_SUPERS)
            ]
            KG = min(kg, KT)  # k-tiles per transpose-psum bank / ACT copyback
            assert KT % KG == 0
            STAGE_DT = mybir.dt.bfloat16 if cast_loads else F32

            if body_parts == "mm":
                # timing ablation: no prep, so give the MMs initialized inputs
                for t in xT + wT:
                    nc.any.memset(t, 0.5)
            # chunk width in source columns: one psum group per chunk when
            # chunk_prep, else the whole K row-block per stage
            KC = KG * P if chunk_prep else K
            N_CHUNKS = K // KC
            KG_PER_CHUNK = KC // (KG * P)  # psum groups per staged chunk

            fixed_stage = None
            if body_parts in ("all_nodma", "prep_nodma"):
                fixed_stage = const.tile([P, KC], STAGE_DT, name="fixed_stage", tag="fixed_stage")
                nc.any.memset(fixed_stage, 0.25)

            def prep_chunk(src_ap, dstT, s, j, c):
                """Transpose+binarize chunk c (KC columns) of row-block j of
                super s into dstT[:, ..., j*P:(j+1)*P] fp8 (+-0.5)."""
                r0 = s * SUPER + j * P
                if fixed_stage is not None:
                    st = fixed_stage
                else:
                    st = stage_pool.tile([P, KC], STAGE_DT, name="stage", tag="stage")
                    if cast_loads:
                        # SWDGE casts f32 -> bf16 inline; sign is preserved
                        # exactly (bf16 keeps the f32 sign+exponent), which is
                        # all the binarize needs
                        nc.gpsimd.dma_start(st, src_ap[r0 : r0 + P, c * KC : (c + 1) * KC])
                    else:
                        nc.sync.dma_start(st, src_ap[r0 : r0 + P, c * KC : (c + 1) * KC])
                bst = stage_pool.tile([P, KC], FP8, name="bst", tag="bst")
                nc.vector.tensor_scalar(
                    out=bst,
                    in0=st,
                    scalar1=0.0,
                    scalar2=0.5,
                    op0=AluOpType.is_ge,
                    op1=AluOpType.subtract,
                )
                for g in range(KG_PER_CHUNK):
                    # fp8 transpose writes PSUM with element step 2
                    ps = tps_pool.tile([P, KG, P, 2], FP8, name="tps", tag="tps")
                    for t in range(KG):
                        nc.tensor.transpose(
                            ps[:, t, :, 0],
                            bst[:, (g * KG + t) * P : (g * KG + t + 1) * P],
                            identity,
                        )
                    kt0 = c * KG_PER_CHUNK * KG + g * KG
                    nc.scalar.activation(
                        dstT[:, kt0 : kt0 + KG, j * P : (j + 1) * P],
                        ps[:, :, :, 0],
                        mybir.ActivationFunctionType.Copy,
                    )

            def mm_group(ms, os_, mt):
                """16 (or 32) accumulating MMs for one [128, SUPER] psum."""
                psum = mm_pool.tile([P, SUPER], F32, name="mmps", tag="mmps")
                if double_row:
                    for kt in range(0, KT, 2):
                        nc.tensor.matmul(
                            psum,
                            lhsT=xT[ms][:, kt : kt + 2, mt * P : (mt + 1) * P],
                            rhs=wT[os_][:, kt : kt + 2, :],
                            start=(kt == 0),
                            stop=(kt == KT - 2),
                            perf_mode=mybir.MatmulPerfMode.DoubleRow,
                        )
                else:
                    for kt in range(KT):
                        nc.tensor.matmul(
                            psum,
                            lhsT=xT[ms][:, kt, mt * P : (mt + 1) * P],
                            rhs=wT[os_][:, kt, :],
                            start=(kt == 0),
                            stop=(kt == KT - 1),
                        )
                return psum

            def epi_group(ms, os_, mt, psum):
                ob = out_pool.tile([P, SUPER], F32, name="ob", tag="ob")
                # psum holds exact_int/4; scale back to exact integers
                nc.scalar.activation(
                    ob, psum, mybir.ActivationFunctionType.Copy, scale=4.0
                )
                nc.vector.tensor_tensor(
                    ob,
                    ob,
                    bias_sb[:, os_ * SUPER : (os_ + 1) * SUPER],
                    AluOpType.add,
                )
                r0 = ms * SUPER + mt * P
                nc.sync.dma_start(
                    out[r0 : r0 + P, os_ * SUPER : (os_ + 1) * SUPER], ob
                )

            def main_block(ms, os_):
                for mt in range(RB):
                    psum = mm_group(ms, os_, mt)
                    epi_group(ms, os_, mt, psum)

            def emit_body():
                if body_parts == "mm":
                    for ms in range(M_SUPERS):
                        for os_ in range(N_SUPERS):
                            main_block(ms, os_)
                    return
                # First x0/w0 supers chunk-major (when chunked) so every
                # operand's first k-chunks land early.
                first = [("x", 0), ("w", 0)]
                rest = [("w", o) for o in range(1, N_SUPERS)] + [
                    ("x", m) for m in range(1, M_SUPERS)
                ]
                prep_q = [
                    (kind, s, j, c)
                    for c in range(N_CHUNKS)
                    for kind, s in first
                    for j in range(RB)
                ] + [
                    (kind, s, j, c)
                    for kind, s in rest
                    for j in range(RB)
                    for c in range(N_CHUNKS)
                ]
                units_per_half = max(1, (4 * N_CHUNKS) // 4)
                if body_parts in ("prep", "prep_nodma"):
                    for kind, s, j, c in prep_q:
                        prep_chunk(
                            x_in if kind == "x" else w_in,
                            xT[s] if kind == "x" else wT[s],
                            s,
                            j,
                            c,
                        )
                    return

                emitted = {"x": set(), "w": set()}

                def emit_prep():
                    kind, s, j, c = prep_q.pop(0)
                    prep_chunk(
                        x_in if kind == "x" else w_in,
                        xT[s] if kind == "x" else wT[s],
                        s,
                        j,
                        c,
                    )
                    emitted[kind].add((s, j, c))

                def deps_met(ms, os_):
                    return all(
                        (ms, j, c) in emitted["x"]
                        for j in range(RB)
                        for c in range(N_CHUNKS)
                    ) and all(
                        (os_, j, c) in emitted["w"]
                        for j in range(RB)
                        for c in range(N_CHUNKS)
                    )

                # Fine-grained software pipeline. Per step: one MM half-block
                # (2 psum groups), one prep row-block unit, then the PREVIOUS
                # half-block's epilogue. Ordering the epilogue after the prep
                # unit keeps next-super DVE binarize / ACT copyback ops ahead
                # of MM-gated epilogue ops in their engine queues, so prep
                # overlaps the MM stream instead of head-of-line blocking it.
                mains = [
                    (ms, os_, half)
                    for ms in range(M_SUPERS)
                    for os_ in range(N_SUPERS)
                    for half in range(RB // 2)
                ]
                pending = None  # (ms, os_, [(mt, psum), ...])
                for ms, os_, half in mains:
                    while prep_q and not deps_met(ms, os_):
                        emit_prep()
                    groups = [
                        (mt, mm_group(ms, os_, mt))
                        for mt in (2 * half, 2 * half + 1)
                    ]
                    # ~1 row-block of prep per half-block paces prep DMA
                    # against the MM stream
                    for _ in range(units_per_half):
                        if prep_q:
                            emit_prep()
                    if pending is not None:
                        pms, pos, pgroups = pending
                        for mt, psum in pgroups:
                            epi_group(pms, pos, mt, psum)
                    pending = (ms, os_, groups)
                if pending is not None:
                    pms, pos, pgroups = pending
                    for mt, psum in pgroups:
                        epi_group(pms, pos, mt, psum)

            if repeat > 1:
                with tc.For_i(0, repeat, 1):
                    emit_body()
            else:
                emit_body()

            if timing_variant:
                dsb = out_pool.tile([P, 16], F32, name="dsb", tag="dsb")
                nc.any.memset(dsb, 1.0)
                nc.sync.dma_start(dummy_out, dsb)

    nc.compile()
    return nc


_NC_CACHE = {}


def _get_nc(M, K, N, double_row=True, prep_mode="fp8t"):
    key = (M, K, N, double_row, prep_mode)
    if key not in _NC_CACHE:
        _NC_CACHE[key] = build_nc(
            M, K, N, double_row=double_row, prep_mode=prep_mode
        )
    return _NC_CACHE[key]


LAST_RESULTS = None


def make_in_maps(x, weight, bias):
    MS = x.shape[0] // M_SPLIT
    NS = weight.shape[0] // N_SPLIT
    in_maps = []
    for c in range(N_CORES):
        mi, ni = divmod(c, N_SPLIT)
        in_maps.append(
            {
                "x_shard": np.ascontiguousarray(x[mi * MS : (mi + 1) * MS]),
                "w_shard": np.ascontiguousarray(weight[ni * NS : (ni + 1) * NS]),
                "bias_rep": np.ascontiguousarray(
                    np.broadcast_to(bias[None, ni * NS : (ni + 1) * NS], (P, NS))
                ),
            }
        )
    return in_maps


def kernel(x, weight, bias):
    global LAST_RESULTS
    x = np.ascontiguousarray(np.asarray(x, dtype=np.float32))
    weight = np.ascontiguousarray(np.asarray(weight, dtype=np.float32))
    bias = np.ascontiguousarray(np.asarray(bias, dtype=np.float32))
    B, K = x.shape
    O = weight.shape[0]
    assert B % M_SPLIT == 0 and O % N_SPLIT == 0

    double_row = os.environ.get("BINLIN_DOUBLE_ROW", "1") == "1"
    prep_mode = os.environ.get("BINLIN_PREP", "fp8t")
    nc = _get_nc(
        B // M_SPLIT, K, O // N_SPLIT, double_row=double_row, prep_mode=prep_mode
    )
    in_maps = make_in_maps(x, weight, bias)

    res = run_bass_kernel_spmd(nc, in_maps, core_ids=list(range(N_CORES)))
    LAST_RESULTS = res

    MS = B // M_SPLIT
    NS = O // N_SPLIT
    out = np.empty((B, O), dtype=np.float32)
    for c in range(N_CORES):
        mi, ni = divmod(c, N_SPLIT)
        out[mi * MS : (mi + 1) * MS, ni * NS : (ni + 1) * NS] = res.results[c][
            "out_shard"
        ]
    return out


# revision 41
# speedup vs baseline: 76438.5099x; 76438.5099x over previous
"""Trainium2 Bass kernel for nn_BinaryLinear (binarized linear layer).

Computes: out = sign(x) @ sign(W).T + bias
  x: [8192, 4096] f32, W: [4096, 4096] f32, bias: [4096] f32 -> out [8192, 4096] f32
  sign(v) = +1 if v >= 0 else -1

Sharding: 4x2 grid over 8 NeuronCores — batch split 4 ways (2048 rows each),
W rows (out_features) split 2 ways (2048 each). Each core computes a disjoint
[2048, 2048] output block; no collectives. This minimizes per-core input bytes
(32 MiB x-shard + 32 MiB W-shard).

Device-side algorithm (per core), all exact (rel err 0 vs the reference):
  1. Prep: SWDGE cast-DMA loads each natural-layout row-block f32 -> bf16
     (sign-exact), DVE binarizes to fp8 +-0.5 in one op ((v>=0) - 0.5), the
     TensorEngine transposes the fp8 128x128 tiles (identity matmul, fast FWL
     weight loads, stride-2 PSUM writes), ACT copies PSUM -> resident SBUF
     operand tensors [K-on-partitions, rows-on-free].
  2. Matmul: fp8 DoubleRow K-accumulated PE matmuls (256-contraction per MM,
     ~95% of fp8 peak), psum = exact_integer_result / 4 (quarter-integers
     bounded by 1024 accumulate exactly in fp32 PSUM).
  3. Epilogue: ACT copy with scale=4 (psum*4 -> exact integers), DVE add of the
     bias row (pre-replicated across 128 partitions on host), DMA out.
Emission is software-pipelined: per step one MM half-block, one prep row-block
unit, then the previous half-block's (MM-gated) epilogue, so prep engine work
never head-of-line blocks behind epilogues and DMA streams continuously.
"""

import os

import numpy as np

import concourse.bacc as bacc
import concourse.mybir as mybir
import concourse.tile as tile
from concourse.alu_op_type import AluOpType
from concourse.bass_utils import run_bass_kernel_spmd
from concourse.masks import make_identity

P = 128
N_CORES = 8
M_SPLIT = 4  # batch split
N_SPLIT = 2  # out_features split

# Full-problem shapes (hardcoded per harness contract)
BATCH = 8192
IN_FEATURES = 4096
OUT_FEATURES = 4096

F32 = mybir.dt.float32
FP8 = mybir.dt.float8e4

SUPER = 512  # rows per prep "super" == matmul o-panel width (one PSUM bank)


def build_nc(
    M,
    K,
    N,
    n_cores=N_CORES,
    double_row=True,
    repeat=1,
    timing_variant=False,
    body_parts="all",  # "all" | "mm" | "prep"  (timing ablation)
    prep_mode="fp8t",  # "fp8t": binarize then fp8 transpose; "f32t": f32 transpose then binarize
    cast_loads=True,  # SWDGE cast-DMA loads (f32 -> bf16 during DMA)
    kg=8,  # k-tiles batched per transpose-psum bank / ACT copyback
    tps_bufs=2,
    mm_bufs=6,
    out_bufs=6,
    stage_bufs=3,
    chunk_prep=False,  # split prep loads into KC-column chunks
):
    """Build the per-core kernel: x_shard [M, K], w_shard [N, K],
    bias_rep [P, N] -> out_shard [M, N].

    repeat/timing_variant are for HW timing only (wrap the body in a device-side
    loop; inputs/outputs become internal DRAM so nothing ships over the tunnel).
    The graded kernel() path always uses repeat=1, timing_variant=False.
    """
    assert M % SUPER == 0 and N % SUPER == 0 and K % P == 0
    KT = K // P  # contraction tiles
    M_SUPERS = M // SUPER
    N_SUPERS = N // SUPER
    RB = SUPER // P  # row-blocks per super (4)
    if double_row:
        assert KT % 2 == 0

    nc = bacc.Bacc(
        "TRN2", target_bir_lowering=False, debug=False, num_devices=n_cores
    )
    if timing_variant:
        x_in = nc.dram_tensor("x_int", [M, K], F32).ap()
        w_in = nc.dram_tensor("w_int", [N, K], F32).ap()
        b_in = nc.dram_tensor("b_int", [P, N], F32).ap()
        out = nc.dram_tensor("out_int", [M, N], F32).ap()
        dummy_out = nc.dram_tensor("dummy_out", [P, 16], F32, kind="ExternalOutput").ap()
    else:
        x_in = nc.dram_tensor("x_shard", [M, K], F32, kind="ExternalInput").ap()
        w_in = nc.dram_tensor("w_shard", [N, K], F32, kind="ExternalInput").ap()
        b_in = nc.dram_tensor("bias_rep", [P, N], F32, kind="ExternalInput").ap()
        out = nc.dram_tensor("out_shard", [M, N], F32, kind="ExternalOutput").ap()

    with tile.TileContext(nc) as tc:
        with (
            tc.tile_pool(name="const", bufs=1) as const,
            tc.tile_pool(name="resid", bufs=1) as resid,
            tc.tile_pool(name="stage", bufs=stage_bufs) as stage_pool,
            tc.tile_pool(name="tps", bufs=tps_bufs, space="PSUM") as tps_pool,
            tc.tile_pool(name="mm", bufs=mm_bufs, space="PSUM") as mm_pool,
            tc.tile_pool(name="outp", bufs=out_bufs) as out_pool,
        ):
            id_dt = FP8 if prep_mode == "fp8t" else F32
            identity = const.tile([P, P], id_dt, name="identity", tag="identity")
            make_identity(nc, identity)
            bias_sb = const.tile([P, N], F32, name="bias_sb", tag="bias_sb")
            nc.sync.dma_start(bias_sb, b_in)

            xT = [
                resid.tile([P, KT, SUPER], FP8, name=f"xT{s}", tag=f"xT{s}")
                for s in range(M_SUPERS)
            ]
            wT = [
                resid.tile([P, KT, SUPER], FP8, name=f"wT{s}", tag=f"wT{s}")
                for s in range(N_SUPERS)
            ]
            KG = min(kg, KT)  # k-tiles per transpose-psum bank / ACT copyback
            assert KT % KG == 0
            STAGE_DT = mybir.dt.bfloat16 if cast_loads else F32

            if body_parts == "mm":
                # timing ablation: no prep, so give the MMs initialized inputs
                for t in xT + wT:
                    nc.any.memset(t, 0.5)
            # chunk width in source columns: one psum group per chunk when
            # chunk_prep, else the whole K row-block per stage
            KC = KG * P if chunk_prep else K
            N_CHUNKS = K // KC
            KG_PER_CHUNK = KC // (KG * P)  # psum groups per staged chunk

            fixed_stage = None
            if body_parts in ("all_nodma", "prep_nodma"):
                fixed_stage = const.tile([P, KC], STAGE_DT, name="fixed_stage", tag="fixed_stage")
                nc.any.memset(fixed_stage, 0.25)

            def prep_chunk(src_ap, dstT, s, j, c):
                """Transpose+binarize chunk c (KC columns) of row-block j of
                super s into dstT[:, ..., j*P:(j+1)*P] fp8 (+-0.5)."""
                r0 = s * SUPER + j * P
                if fixed_stage is not None:
                    st = fixed_stage
                else:
                    st = stage_pool.tile([P, KC], STAGE_DT, name="stage", tag="stage")
                    if cast_loads:
                        # SWDGE casts f32 -> bf16 inline; sign is preserved
                        # exactly (bf16 keeps the f32 sign+exponent), which is
                        # all the binarize needs
                        nc.gpsimd.dma_start(st, src_ap[r0 : r0 + P, c * KC : (c + 1) * KC])
                    else:
                        nc.sync.dma_start(st, src_ap[r0 : r0 + P, c * KC : (c + 1) * KC])
                bst = stage_pool.tile([P, KC], FP8, name="bst", tag="bst")
                nc.vector.tensor_scalar(
                    out=bst,
                    in0=st,
                    scalar1=0.0,
                    scalar2=0.5,
                    op0=AluOpType.is_ge,
                    op1=AluOpType.subtract,
                )
                for g in range(KG_PER_CHUNK):
                    # fp8 transpose writes PSUM with element step 2
                    ps = tps_pool.tile([P, KG, P, 2], FP8, name="tps", tag="tps")
                    for t in range(KG):
                        nc.tensor.transpose(
                            ps[:, t, :, 0],
                            bst[:, (g * KG + t) * P : (g * KG + t + 1) * P],
                            identity,
                        )
                    kt0 = c * KG_PER_CHUNK * KG + g * KG
                    nc.scalar.activation(
                        dstT[:, kt0 : kt0 + KG, j * P : (j + 1) * P],
                        ps[:, :, :, 0],
                        mybir.ActivationFunctionType.Copy,
                    )

            def mm_group(ms, os_, mt):
                """16 (or 32) accumulating MMs for one [128, SUPER] psum."""
                psum = mm_pool.tile([P, SUPER], F32, name="mmps", tag="mmps")
                if double_row:
                    for kt in range(0, KT, 2):
                        nc.tensor.matmul(
                            psum,
                            lhsT=xT[ms][:, kt : kt + 2, mt * P : (mt + 1) * P],
                            rhs=wT[os_][:, kt : kt + 2, :],
                            start=(kt == 0),
                            stop=(kt == KT - 2),
                            perf_mode=mybir.MatmulPerfMode.DoubleRow,
                        )
                else:
                    for kt in range(KT):
                        nc.tensor.matmul(
                            psum,
                            lhsT=xT[ms][:, kt, mt * P : (mt + 1) * P],
                            rhs=wT[os_][:, kt, :],
                            start=(kt == 0),
                            stop=(kt == KT - 1),
                        )
                return psum

            def epi_group(ms, os_, mt, psum):
                ob = out_pool.tile([P, SUPER], F32, name="ob", tag="ob")
                # psum holds exact_int/4; scale back to exact integers
                nc.scalar.activation(
                    ob, psum, mybir.ActivationFunctionType.Copy, scale=4.0
                )
                nc.vector.tensor_tensor(
                    ob,
                    ob,
                    bias_sb[:, os_ * SUPER : (os_ + 1) * SUPER],
                    AluOpType.add,
                )
                r0 = ms * SUPER + mt * P
                nc.sync.dma_start(
                    out[r0 : r0 + P, os_ * SUPER : (os_ + 1) * SUPER], ob
                )

            def main_block(ms, os_):
                for mt in range(RB):
                    psum = mm_group(ms, os_, mt)
                    epi_group(ms, os_, mt, psum)

            def emit_body():
                if body_parts == "mm":
                    for ms in range(M_SUPERS):
                        for os_ in range(N_SUPERS):
                            main_block(ms, os_)
                    return
                # First x0/w0 supers chunk-major (when chunked) so every
                # operand's first k-chunks land early.
                first = [("x", 0), ("w", 0)]
                rest = [("w", o) for o in range(1, N_SUPERS)] + [
                    ("x", m) for m in range(1, M_SUPERS)
                ]
                prep_q = [
                    (kind, s, j, c)
                    for c in range(N_CHUNKS)
                    for kind, s in first
                    for j in range(RB)
                ] + [
                    (kind, s, j, c)
                    for kind, s in rest
                    for j in range(RB)
                    for c in range(N_CHUNKS)
                ]
                units_per_half = max(1, (4 * N_CHUNKS) // 4)
                if body_parts in ("prep", "prep_nodma"):
                    for kind, s, j, c in prep_q:
                        prep_chunk(
                            x_in if kind == "x" else w_in,
                            xT[s] if kind == "x" else wT[s],
                            s,
                            j,
                            c,
                        )
                    return

                emitted = {"x": set(), "w": set()}

                def emit_prep():
                    kind, s, j, c = prep_q.pop(0)
                    prep_chunk(
                        x_in if kind == "x" else w_in,
                        xT[s] if kind == "x" else wT[s],
                        s,
                        j,
                        c,
                    )
                    emitted[kind].add((s, j, c))

                def deps_met(ms, os_):
                    return all(
                        (ms, j, c) in emitted["x"]
                        for j in range(RB)
                        for c in range(N_CHUNKS)
                    ) and all(
                        (os_, j, c) in emitted["w"]
                        for j in range(RB)
                        for c in range(N_CHUNKS)
                    )

                # Fine-grained software pipeline. Per step: one MM half-block
                # (2 psum groups), one prep row-block unit, then the PREVIOUS
                # half-block's epilogue. Ordering the epilogue after the prep
                # unit keeps next-super DVE binarize / ACT copyback ops ahead
                # of MM-gated epilogue ops in their engine queues, so prep
                # overlaps the MM stream instead of head-of-line blocking it.
                mains = [
                    (ms, os_, half)
                    for ms in range(M_SUPERS)
                    for os_ in range(N_SUPERS)
                    for half in range(RB // 2)
                ]
                pending = None  # (ms, os_, [(mt, psum), ...])
                for ms, os_, half in mains:
                    while prep_q and not deps_met(ms, os_):
                        emit_prep()
                    groups = [
                        (mt, mm_group(ms, os_, mt))
                        for mt in (2 * half, 2 * half + 1)
                    ]
                    # ~1 row-block of prep per half-block paces prep DMA
                    # against the MM stream
                    for _ in range(units_per_half):
                        if prep_q:
                            emit_prep()
                    if pending is not None:
                        pms, pos, pgroups = pending
                        for mt, psum in pgroups:
                            epi_group(pms, pos, mt, psum)
                    pending = (ms, os_, groups)
                if pending is not None:
                    pms, pos, pgroups = pending
                    for mt, psum in pgroups:
                        epi_group(pms, pos, mt, psum)

            if repeat > 1:
                with tc.For_i(0, repeat, 1):
                    emit_body()
            else:
                emit_body()

            if timing_variant:
                dsb = out_pool.tile([P, 16], F32, name="dsb", tag="dsb")
                nc.any.memset(dsb, 1.0)
                nc.sync.dma_start(dummy_out, dsb)

    nc.compile()
    return nc


_NC_CACHE = {}


def _get_nc(M, K, N, double_row=True, prep_mode="fp8t"):
    key = (M, K, N, double_row, prep_mode)
    if key not in _NC_CACHE:
        _NC_CACHE[key] = build_nc(
            M, K, N, double_row=double_row, prep_mode=prep_mode
        )
    return _NC_CACHE[key]


LAST_RESULTS = None


def make_in_maps(x, weight, bias):
    MS = x.shape[0] // M_SPLIT
    NS = weight.shape[0] // N_SPLIT
    in_maps = []
    for c in range(N_CORES):
        mi, ni = divmod(c, N_SPLIT)
        in_maps.append(
            {
                "x_shard": np.ascontiguousarray(x[mi * MS : (mi + 1) * MS]),
                "w_shard": np.ascontiguousarray(weight[ni * NS : (ni + 1) * NS]),
                "bias_rep": np.ascontiguousarray(
                    np.broadcast_to(bias[None, ni * NS : (ni + 1) * NS], (P, NS))
                ),
            }
        )
    return in_maps


def kernel(x, weight, bias):
    global LAST_RESULTS
    x = np.ascontiguousarray(np.asarray(x, dtype=np.float32))
    weight = np.ascontiguousarray(np.asarray(weight, dtype=np.float32))
    bias = np.ascontiguousarray(np.asarray(bias, dtype=np.float32))
    B, K = x.shape
    O = weight.shape[0]
    assert B % M_SPLIT == 0 and O % N_SPLIT == 0

    double_row = os.environ.get("BINLIN_DOUBLE_ROW", "1") == "1"
    prep_mode = os.environ.get("BINLIN_PREP", "fp8t")
    nc = _get_nc(
        B // M_SPLIT, K, O // N_SPLIT, double_row=double_row, prep_mode=prep_mode
    )
    in_maps = make_in_maps(x, weight, bias)

    res = run_bass_kernel_spmd(nc, in_maps, core_ids=list(range(N_CORES)))
    LAST_RESULTS = res

    MS = B // M_SPLIT
    NS = O // N_SPLIT
    out = np.empty((B, O), dtype=np.float32)
    for c in range(N_CORES):
        mi, ni = divmod(c, N_SPLIT)
        out[mi * MS : (mi + 1) * MS, ni * NS : (ni + 1) * NS] = res.results[c][
            "out_shard"
        ]
    return out


# revision 49
# speedup vs baseline: 95000.6413x; 1.2428x over previous
"""Trainium2 Bass kernel for nn_BinaryLinear (binarized linear layer).

Computes: out = sign(x) @ sign(W).T + bias
  x: [8192, 4096] f32, W: [4096, 4096] f32, bias: [4096] f32 -> out [8192, 4096] f32
  sign(v) = +1 if v >= 0 else -1

Sharding: 4x2 grid over 8 NeuronCores — batch split 4 ways (2048 rows each),
W rows (out_features) split 2 ways (2048 each). Each core computes a disjoint
[2048, 2048] output block; no collectives. This minimizes per-core input bytes
(32 MiB x-shard + 32 MiB W-shard).

Device-side algorithm (per core), all exact (rel err 0 vs the reference):
  1. Prep: SWDGE cast-DMA loads each natural-layout row-block f32 -> bf16
     (sign-exact), DVE binarizes to fp8 +-0.5 in one op ((v>=0) - 0.5), the
     TensorEngine transposes the fp8 128x128 tiles (identity matmul, fast FWL
     weight loads, stride-2 PSUM writes), ACT copies PSUM -> resident SBUF
     operand tensors [K-on-partitions, rows-on-free].
  2. Matmul: fp8 DoubleRow K-accumulated PE matmuls (256-contraction per MM,
     ~95% of fp8 peak), psum = exact_integer_result / 4 (quarter-integers
     bounded by 1024 accumulate exactly in fp32 PSUM).
  3. Epilogue: ACT copy with scale=4 (psum*4 -> exact integers), DVE add of the
     bias row (pre-replicated across 128 partitions on host), DMA out.
Emission is software-pipelined: per step one MM half-block, one prep row-block
unit, then the previous half-block's (MM-gated) epilogue, so prep engine work
never head-of-line blocks behind epilogues and DMA streams continuously.
"""

import os

import numpy as np

import concourse.bacc as bacc
import concourse.mybir as mybir
import concourse.tile as tile
from concourse.alu_op_type import AluOpType
from concourse.bass_utils import run_bass_kernel_spmd
from concourse.masks import make_identity

P = 128
N_CORES = 8
M_SPLIT = 4  # batch split
N_SPLIT = 2  # out_features split

# Full-problem shapes (hardcoded per harness contract)
BATCH = 8192
IN_FEATURES = 4096
OUT_FEATURES = 4096

F32 = mybir.dt.float32
FP8 = mybir.dt.float8e4

SUPER = 512  # rows per prep "super" == matmul o-panel width (one PSUM bank)


def build_nc(
    M,
    K,
    N,
    n_cores=N_CORES,
    double_row=True,
    repeat=1,
    timing_variant=False,
    body_parts="all",  # "all" | "mm" | "prep"  (timing ablation)
    prep_mode="fp8t",  # "fp8t": binarize then fp8 transpose; "f32t": f32 transpose then binarize
    cast_loads=True,  # SWDGE cast-DMA loads (f32 -> bf16 during DMA)
    kg=8,  # k-tiles batched per transpose-psum bank / ACT copyback
    tps_bufs=2,
    mm_bufs=6,
    out_bufs=6,
    stage_bufs=3,
    chunk_prep=False,  # split ALL prep loads into KC-column chunks
    first_chunks=False,  # chunk-major k-split staging for the first x0/w0 supers
):
    """Build the per-core kernel: x_shard [M, K], w_shard [N, K],
    bias_rep [P, N] -> out_shard [M, N].

    repeat/timing_variant are for HW timing only (wrap the body in a device-side
    loop; inputs/outputs become internal DRAM so nothing ships over the tunnel).
    The graded kernel() path always uses repeat=1, timing_variant=False.
    """
    assert M % SUPER == 0 and N % SUPER == 0 and K % P == 0
    KT = K // P  # contraction tiles
    M_SUPERS = M // SUPER
    N_SUPERS = N // SUPER
    RB = SUPER // P  # row-blocks per super (4)
    if double_row:
        assert KT % 2 == 0

    nc = bacc.Bacc(
        "TRN2", target_bir_lowering=False, debug=False, num_devices=n_cores
    )
    if timing_variant:
        x_in = nc.dram_tensor("x_int", [M, K], F32).ap()
        w_in = nc.dram_tensor("w_int", [N, K], F32).ap()
        b_in = nc.dram_tensor("b_int", [P, N], F32).ap()
        out = nc.dram_tensor("out_int", [M, N], F32).ap()
        dummy_out = nc.dram_tensor("dummy_out", [P, 16], F32, kind="ExternalOutput").ap()
    else:
        x_in = nc.dram_tensor("x_shard", [M, K], F32, kind="ExternalInput").ap()
        w_in = nc.dram_tensor("w_shard", [N, K], F32, kind="ExternalInput").ap()
        b_in = nc.dram_tensor("bias_rep", [P, N], F32, kind="ExternalInput").ap()
        out = nc.dram_tensor("out_shard", [M, N], F32, kind="ExternalOutput").ap()

    with tile.TileContext(nc) as tc:
        with (
            tc.tile_pool(name="const", bufs=1) as const,
            tc.tile_pool(name="resid", bufs=1) as resid,
            tc.tile_pool(name="stage", bufs=stage_bufs) as stage_pool,
            tc.tile_pool(name="tps", bufs=tps_bufs, space="PSUM") as tps_pool,
            tc.tile_pool(name="mm", bufs=mm_bufs, space="PSUM") as mm_pool,
            tc.tile_pool(name="outp", bufs=out_bufs) as out_pool,
        ):
            id_dt = FP8 if prep_mode == "fp8t" else F32
            identity = const.tile([P, P], id_dt, name="identity", tag="identity")
            make_identity(nc, identity)
            bias_sb = const.tile([P, N], F32, name="bias_sb", tag="bias_sb")
            nc.sync.dma_start(bias_sb, b_in)

            xT = [
                resid.tile([P, KT, SUPER], FP8, name=f"xT{s}", tag=f"xT{s}")
                for s in range(M_SUPERS)
            ]
            wT = [
                resid.tile([P, KT, SUPER], FP8, name=f"wT{s}", tag=f"wT{s}")
                for s in range(N_SUPERS)
            ]
            KG = min(kg, KT)  # k-tiles per transpose-psum bank / ACT copyback
            assert KT % KG == 0
            STAGE_DT = mybir.dt.bfloat16 if cast_loads else F32

            if body_parts == "mm":
                # timing ablation: no prep, so give the MMs initialized inputs
                for t in xT + wT:
                    nc.any.memset(t, 0.5)
            # chunk width in source columns: one psum group per chunk when
            # chunk_prep, else the whole K row-block per stage
            KC = KG * P if chunk_prep else K
            N_CHUNKS = K // KC
            KG_PER_CHUNK = KC // (KG * P)  # psum groups per staged chunk

            fixed_stage = None
            if body_parts in ("all_nodma", "prep_nodma"):
                fixed_stage = const.tile([P, KC], STAGE_DT, name="fixed_stage", tag="fixed_stage")
                nc.any.memset(fixed_stage, 0.25)

            def prep_chunk(src_ap, dstT, s, j, c, small=False):
                """Transpose+binarize chunk c of row-block j of super s into
                dstT[:, ..., j*P:(j+1)*P] fp8 (+-0.5). small=True stages one
                KG*P-column chunk (startup path); else KC columns."""
                kc = KG * P if small else KC
                groups = kc // (KG * P)
                tag = "stageC" if small else "stage"
                r0 = s * SUPER + j * P
                if fixed_stage is not None and not small:
                    st = fixed_stage
                else:
                    st = stage_pool.tile(
                        [P, kc], STAGE_DT, name=tag, tag=tag,
                        bufs=6 if small else None,
                    )
                    if cast_loads:
                        # SWDGE casts f32 -> bf16 inline; sign is preserved
                        # exactly (bf16 keeps the f32 sign+exponent), which is
                        # all the binarize needs
                        nc.gpsimd.dma_start(st, src_ap[r0 : r0 + P, c * kc : (c + 1) * kc])
                    else:
                        nc.sync.dma_start(st, src_ap[r0 : r0 + P, c * kc : (c + 1) * kc])
                btag = "bstC" if small else "bst"
                bst = stage_pool.tile(
                    [P, kc], FP8, name=btag, tag=btag, bufs=6 if small else None
                )
                nc.vector.tensor_scalar(
                    out=bst,
                    in0=st,
                    scalar1=0.0,
                    scalar2=0.5,
                    op0=AluOpType.is_ge,
                    op1=AluOpType.subtract,
                )
                for g in range(groups):
                    # fp8 transpose writes PSUM with element step 2
                    ps = tps_pool.tile([P, KG, P, 2], FP8, name="tps", tag="tps")
                    for t in range(KG):
                        nc.tensor.transpose(
                            ps[:, t, :, 0],
                            bst[:, (g * KG + t) * P : (g * KG + t + 1) * P],
                            identity,
                        )
                    kt0 = c * groups * KG + g * KG
                    nc.scalar.activation(
                        dstT[:, kt0 : kt0 + KG, j * P : (j + 1) * P],
                        ps[:, :, :, 0],
                        mybir.ActivationFunctionType.Copy,
                    )

            def mm_group(ms, os_, mt):
                """16 (or 32) accumulating MMs for one [128, SUPER] psum."""
                psum = mm_pool.tile([P, SUPER], F32, name="mmps", tag="mmps")
                if double_row:
                    for kt in range(0, KT, 2):
                        nc.tensor.matmul(
                            psum,
                            lhsT=xT[ms][:, kt : kt + 2, mt * P : (mt + 1) * P],
                            rhs=wT[os_][:, kt : kt + 2, :],
                            start=(kt == 0),
                            stop=(kt == KT - 2),
                            perf_mode=mybir.MatmulPerfMode.DoubleRow,
                        )
                else:
                    for kt in range(KT):
                        nc.tensor.matmul(
                            psum,
                            lhsT=xT[ms][:, kt, mt * P : (mt + 1) * P],
                            rhs=wT[os_][:, kt, :],
                            start=(kt == 0),
                            stop=(kt == KT - 1),
                        )
                return psum

            def epi_group(ms, os_, mt, psum):
                ob = out_pool.tile([P, SUPER], F32, name="ob", tag="ob")
                # psum holds exact_int/4; scale back to exact integers
                nc.scalar.activation(
                    ob, psum, mybir.ActivationFunctionType.Copy, scale=4.0
                )
                nc.vector.tensor_tensor(
                    ob,
                    ob,
                    bias_sb[:, os_ * SUPER : (os_ + 1) * SUPER],
                    AluOpType.add,
                )
                r0 = ms * SUPER + mt * P
                nc.sync.dma_start(
                    out[r0 : r0 + P, os_ * SUPER : (os_ + 1) * SUPER], ob
                )

            def main_block(ms, os_):
                for mt in range(RB):
                    psum = mm_group(ms, os_, mt)
                    epi_group(ms, os_, mt, psum)

            def emit_body():
                if body_parts == "mm":
                    for ms in range(M_SUPERS):
                        for os_ in range(N_SUPERS):
                            main_block(ms, os_)
                    return
                # First x0/w0 supers: chunk-major small-chunk staging so every
                # operand's first k-groups land after ~4 MiB of DMA and the
                # scheduler can start MMs ~30us earlier. Steady state keeps
                # efficient full-K row-block loads.
                SMALL_CHUNKS = KT // KG  # small chunks per row-block
                first = [("x", 0), ("w", 0)]
                rest = [("w", o) for o in range(1, N_SUPERS)] + [
                    ("x", m) for m in range(1, M_SUPERS)
                ]
                if first_chunks and SMALL_CHUNKS > 1:
                    # (kind, s, j, chunk, small, weight)
                    first_q = [
                        (kind, s, j, c, True, 1)
                        for c in range(SMALL_CHUNKS)
                        for kind, s in first
                        for j in range(RB)
                    ]
                else:
                    first_q = [
                        (kind, s, j, c, False, SMALL_CHUNKS // N_CHUNKS)
                        for kind, s in first
                        for j in range(RB)
                        for c in range(N_CHUNKS)
                    ]
                prep_q = first_q + [
                    (kind, s, j, c, False, SMALL_CHUNKS // N_CHUNKS)
                    for kind, s in rest
                    for j in range(RB)
                    for c in range(N_CHUNKS)
                ]
                totals = {"x": {}, "w": {}}
                for kind, s, j, c, small, wgt in prep_q:
                    totals[kind][s] = totals[kind].get(s, 0) + 1
                if body_parts in ("prep", "prep_nodma"):
                    for kind, s, j, c, small, wgt in prep_q:
                        prep_chunk(
                            x_in if kind == "x" else w_in,
                            xT[s] if kind == "x" else wT[s],
                            s,
                            j,
                            c,
                            small,
                        )
                    return

                done = {"x": {}, "w": {}}

                def emit_prep():
                    kind, s, j, c, small, wgt = prep_q.pop(0)
                    prep_chunk(
                        x_in if kind == "x" else w_in,
                        xT[s] if kind == "x" else wT[s],
                        s,
                        j,
                        c,
                        small,
                    )
                    done[kind][s] = done[kind].get(s, 0) + 1
                    return wgt

                def deps_met(ms, os_):
                    return done["x"].get(ms, 0) == totals["x"][ms] and done[
                        "w"
                    ].get(os_, 0) == totals["w"][os_]

                # Fine-grained software pipeline. Per step: one MM half-block
                # (2 psum groups), one prep row-block unit, then the PREVIOUS
                # half-block's epilogue. Ordering the epilogue after the prep
                # unit keeps next-super DVE binarize / ACT copyback ops ahead
                # of MM-gated epilogue ops in their engine queues, so prep
                # overlaps the MM stream instead of head-of-line blocking it.
                mains = [
                    (ms, os_, half)
                    for ms in range(M_SUPERS)
                    for os_ in range(N_SUPERS)
                    for half in range(RB // 2)
                ]
                pending = None  # (ms, os_, [(mt, psum), ...])
                for ms, os_, half in mains:
                    while prep_q and not deps_met(ms, os_):
                        emit_prep()
                    groups = [
                        (mt, mm_group(ms, os_, mt))
                        for mt in (2 * half, 2 * half + 1)
                    ]
                    # ~1 row-block-equivalent of prep per half-block paces
                    # prep DMA against the MM stream
                    want = SMALL_CHUNKS
                    while prep_q and want > 0:
                        want -= emit_prep()
                    if pending is not None:
                        pms, pos, pgroups = pending
                        for mt, psum in pgroups:
                            epi_group(pms, pos, mt, psum)
                    pending = (ms, os_, groups)
                if pending is not None:
                    pms, pos, pgroups = pending
                    for mt, psum in pgroups:
                        epi_group(pms, pos, mt, psum)

            if repeat > 1:
                with tc.For_i(0, repeat, 1):
                    emit_body()
            else:
                emit_body()

            if timing_variant:
                dsb = out_pool.tile([P, 16], F32, name="dsb", tag="dsb")
                nc.any.memset(dsb, 1.0)
                nc.sync.dma_start(dummy_out, dsb)

    nc.compile()
    return nc


_NC_CACHE = {}


def _get_nc(M, K, N, double_row=True, prep_mode="fp8t"):
    key = (M, K, N, double_row, prep_mode)
    if key not in _NC_CACHE:
        _NC_CACHE[key] = build_nc(
            M, K, N, double_row=double_row, prep_mode=prep_mode
        )
    return _NC_CACHE[key]


LAST_RESULTS = None


def make_in_maps(x, weight, bias):
    MS = x.shape[0] // M_SPLIT
    NS = weight.shape[0] // N_SPLIT
    in_maps = []
    for c in range(N_CORES):
        mi, ni = divmod(c, N_SPLIT)
        in_maps.append(
            {
                "x_shard": np.ascontiguousarray(x[mi * MS : (mi + 1) * MS]),
                "w_shard": np.ascontiguousarray(weight[ni * NS : (ni + 1) * NS]),
                "bias_rep": np.ascontiguousarray(
                    np.broadcast_to(bias[None, ni * NS : (ni + 1) * NS], (P, NS))
                ),
            }
        )
    return in_maps


def kernel(x, weight, bias):
    global LAST_RESULTS
    x = np.ascontiguousarray(np.asarray(x, dtype=np.float32))
    weight = np.ascontiguousarray(np.asarray(weight, dtype=np.float32))
    bias = np.ascontiguousarray(np.asarray(bias, dtype=np.float32))
    B, K = x.shape
    O = weight.shape[0]
    assert B % M_SPLIT == 0 and O % N_SPLIT == 0

    double_row = os.environ.get("BINLIN_DOUBLE_ROW", "1") == "1"
    prep_mode = os.environ.get("BINLIN_PREP", "fp8t")
    nc = _get_nc(
        B // M_SPLIT, K, O // N_SPLIT, double_row=double_row, prep_mode=prep_mode
    )
    in_maps = make_in_maps(x, weight, bias)

    last_exc = None
    for _attempt in range(3):
        try:
            res = run_bass_kernel_spmd(nc, in_maps, core_ids=list(range(N_CORES)))
            break
        except Exception as e:  # transient NRT/device wedges recover on retry
            last_exc = e
            os.environ.setdefault("NEURON_RT_RESET_CORES", "1")
    else:
        raise last_exc
    LAST_RESULTS = res

    MS = B // M_SPLIT
    NS = O // N_SPLIT
    out = np.empty((B, O), dtype=np.float32)
    for c in range(N_CORES):
        mi, ni = divmod(c, N_SPLIT)
        out[mi * MS : (mi + 1) * MS, ni * NS : (ni + 1) * NS] = res.results[c][
            "out_shard"
        ]
    return out
